# revision 14
# baseline (speedup 1.0000x reference)
"""Trainium2 Bass kernel for a local-attention layer (GQA + RoPE + banded mask).

Full computation (reference semantics, f32):
  q = x@wq, k = x@wk, v = x@wv  (B=2, S=2048, D=2048, Hq=16, Hkv=4, hd=128)
  rope(q), rope(k) interleaved-pair style
  banded causal attention, window=1024, softmax
  out = (probs @ v_rep) @ wo

Sharding: 8 cores = (batch b in {0,1}) x (kv-group g in {0..3}).
Core c handles batch c//4, kv head c%4 and its 4 q heads.  Each core
computes a partial (2048, 2048) f16 output (its heads' contribution
through wo rows); host sums the 4 partials per batch in f32.

v1 design (all matmuls f16, f32 PSUM accum; PE kept warm):
  - X^T is precomputed on the HOST (f16), so no PE transposes are needed
    for the projections; QT/KT/VT = W^T X^T directly.
  - V natural layout obtained via DMA-xbar transpose (scalar-engine HWDGE),
    not the PE.
  - RoPE in half-split form (host pre-permutes wq/wk columns); computed
    with 4 vector ops per tile using two trig tables [cos;sin] and
    [sin;cos].
  - Scores computed transposed ST[j,i] = KT_tile^T @ QT as (128,512)
    singles; exp on ScalarE (no max subtraction -- scores are provably
    small); banded mask applied post-exp on f16 tiles with pair-merged
    gpsimd.affine_select (one call per homogeneous pair).
  - Softmax denominator via ones-vector matmul into the second PSUM bank
    of the per-head pvdn tile; reciprocal_approx_fast + partition
    broadcast; normalization fused into PSUM->SBUF copy of pv.
  - o-proj accumulates over the 4 heads; f16 output DMA.
  - PSUM plan (8 banks): tag mm x2 two-bank slots [proj tiles use one
    bank; paired score tiles use both -- one exp ACTIVATE per pair],
    tag pv x2 [attention pv + oproj], tag dn x2 (all-ones denominator
    matmul, M=1 matmuls are slow).
  - Chunk loop is software-pipelined [attn(c-1)][proj(c)][oproj(c-1)] so
    the PE never sits idle during softmax post-processing.
"""

import os
import numpy as np

B, S, D = 2, 2048, 2048
NH, NKV, HD = 16, 4, 128
WINDOW = 1024
ROPE_THETA = 10000.0
HQ = NH // NKV          # q heads per core = 4
QD = HQ * HD            # 512
NK = D // 128           # 16 contraction chunks
CH = 512                # s-chunk size
NCH = S // CH           # 4 chunks

_cache = {}


def _host_prep(wq, wk, wv, wo):
    """Per-core weight slices with rope permutation + scale folded in."""
    # de-interleave permutation: dev col j <- ref col (2j if j<64 else 2(j-64)+1)
    perm = np.empty(HD, dtype=np.int64)
    perm[:64] = np.arange(64) * 2
    perm[64:] = np.arange(64) * 2 + 1

    scale = 1.0 / np.sqrt(np.float32(HD))
    wq_p = wq.reshape(D, NH, HD)[:, :, perm].reshape(D, NH * HD) * scale
    wk_p = wk.reshape(D, NKV, HD)[:, :, perm].reshape(D, NKV * HD)

    inv_freq = 1.0 / (ROPE_THETA ** (np.arange(0, HD, 2, dtype=np.float32) / HD))
    t = np.arange(S, dtype=np.float32)
    ang = np.outer(t, inv_freq)             # (S, 64)
    cosT = np.ascontiguousarray(np.cos(ang).T).astype(np.float32)  # (64, S)
    sinT = np.ascontiguousarray(np.sin(ang).T).astype(np.float32)
    # trigC = [cos; cos], trigD = [-sin; sin]: with qf=[lo;hi], qg=[hi;lo],
    # rope(q) = qf*trigC + qg*trigD (every op pairs equal base partitions).
    trigC = np.ascontiguousarray(np.concatenate([cosT, cosT], axis=0)).astype(np.float16)
    trigD = np.ascontiguousarray(np.concatenate([-sinT, sinT], axis=0)).astype(np.float16)

    shards = []
    for c in range(8):
        g = c % 4
        shards.append(dict(
            wq=np.ascontiguousarray(wq_p[:, g * QD:(g + 1) * QD]).astype(np.float16),
            wk=np.ascontiguousarray(wk_p[:, g * HD:(g + 1) * HD]).astype(np.float16),
            wv=np.ascontiguousarray(wv[:, g * HD:(g + 1) * HD]).astype(np.float16),
            wo=np.ascontiguousarray(wo[g * QD:(g + 1) * QD, :]).astype(np.float16),
        ))
    return shards, trigC, trigD


def build_kernel():
    import concourse.bass as bass
    import concourse.mybir as mybir
    import concourse.tile as tile
    from concourse import bacc

    f16 = mybir.dt.float16
    f32 = mybir.dt.float32
    EXP = mybir.ActivationFunctionType.Exp
    GE = mybir.AluOpType.is_ge
    MUL = mybir.AluOpType.mult

    nc = bacc.Bacc("TRN2", target_bir_lowering=False, debug=False, num_devices=8)

    xt_d = nc.dram_tensor("xT", [D, S], f16, kind="ExternalInput").ap()
    wq_d = nc.dram_tensor("wq", [D, QD], f16, kind="ExternalInput").ap()
    wk_d = nc.dram_tensor("wk", [D, HD], f16, kind="ExternalInput").ap()
    wv_d = nc.dram_tensor("wv", [D, HD], f16, kind="ExternalInput").ap()
    wo_d = nc.dram_tensor("wo", [QD, D], f16, kind="ExternalInput").ap()
    trigC_d = nc.dram_tensor("trigC", [128, S], f16, kind="ExternalInput").ap()
    trigD_d = nc.dram_tensor("trigD", [128, S], f16, kind="ExternalInput").ap()
    one_d = nc.dram_tensor("ones", [128, 128], f16, kind="ExternalInput").ap()
    out_d = nc.dram_tensor("out", [S, D], f16, kind="ExternalOutput").ap()

    with tile.TileContext(nc) as tc:
        with (
            tc.tile_pool(name="persist", bufs=1) as pp,
            tc.tile_pool(name="xtp", bufs=NCH) as xp,
            tc.tile_pool(name="qtp", bufs=6) as qtp,
            tc.tile_pool(name="vsp", bufs=2) as vsp,
            tc.tile_pool(name="ptp", bufs=3) as ptp,
            tc.tile_pool(name="ropet", bufs=8) as rtp,
            tc.tile_pool(name="small", bufs=3) as smp,
            tc.tile_pool(name="atp", bufs=5) as atp,
            tc.tile_pool(name="obp", bufs=2) as obp,
            tc.tile_pool(name="psMM", bufs=2, space="PSUM") as psMM,
            tc.tile_pool(name="psPV", bufs=2, space="PSUM") as psPV,
        ):
            # ---- persistent SBUF tensors -------------------------------
            wq_sb = pp.tile([128, NK * QD], f16, tag="wq")      # [k][:, qd]
            wk_sb = pp.tile([128, NK * HD], f16, tag="wk")
            wv_sb = pp.tile([128, NK * HD], f16, tag="wv")
            wo_sb = pp.tile([128, HQ * D], f16, tag="wo")       # [h][:, e]
            trigC_sb = pp.tile([128, S], f16, tag="trigC")  # [cos;cos]
            trigD_sb = pp.tile([128, S], f16, tag="trigD")  # [-sin;sin]
            one_sb = pp.tile([128, 128], f16, tag="one")
            kt_sb = pp.tile([128, S], f16, tag="kt")            # rope'd K^T
            v_sb = pp.tile([128, S], f16, tag="v")              # [jt][s, d]

            # ---- input DMAs (sync engine, in consumption order) --------
            nc.sync.dma_start(one_sb[:], one_d)
            nc.sync.dma_start(trigC_sb[:], trigC_d)
            nc.sync.dma_start(
                wk_sb[:].rearrange("p (k n) -> p k n", k=NK),
                wk_d.rearrange("(k p) n -> p k n", p=128))

            xts = [None] * NCH

            def load_x_chunk(c):
                t_ = xp.tile([128, NK * CH], f16, tag="xt", name=f"xt{c}")
                for q in range(4):
                    nc.sync.dma_start(
                        t_[:, q * 4 * CH:(q + 1) * 4 * CH].rearrange(
                            "p (k s) -> p k s", k=4),
                        xt_d[q * 4 * 128:(q + 1) * 4 * 128,
                             c * CH:(c + 1) * CH].rearrange(
                            "(k p) s -> p k s", p=128))
                xts[c] = t_

            load_x_chunk(0)
            nc.sync.dma_start(trigD_sb[:], trigD_d)
            nc.sync.dma_start(
                wv_sb[:].rearrange("p (k n) -> p k n", k=NK),
                wv_d.rearrange("(k p) n -> p k n", p=128))
            nc.sync.dma_start(
                wq_sb[:].rearrange("p (k n) -> p k n", k=NK),
                wq_d.rearrange("(k p) n -> p k n", p=128))
            load_x_chunk(1)
            nc.sync.dma_start(
                wo_sb[:].rearrange("p (h n) -> p h n", h=HQ),
                wo_d.rearrange("(h p) n -> p h n", p=128))
            load_x_chunk(2)
            load_x_chunk(3)

            # PE warm-up: build HAM activity while the first x/w DMAs land
            # so the real matmuls start at K=8/8 instead of 1.2 GHz.
            warm_ps = psMM.tile([128, CH], f32, tag="mm", name="warmup")
            for _ in range(11):
                nc.tensor.matmul(warm_ps[:], one_sb[:], trigC_sb[:, 0:CH],
                                 start=True, stop=True)

            def rope(dst, src_ps, c):
                """src_ps (128, CH) psum -> dst (128, CH), half-split rope:
                dst = [lo*cos - hi*sin ; lo*sin + hi*cos].
                qf=[lo;hi] (scalar copy, frees the PSUM slot early), then
                qg=[hi;lo] via two single-input copies; all two-input ops
                pair equal base partitions and run f16 2x on the DVE."""
                qf = rtp.tile([128, CH], f16, tag="qf", name="qf")
                nc.vector.tensor_copy(qf[:], src_ps)
                qg = rtp.tile([128, CH], f16, tag="qf", name="qg")
                nc.vector.tensor_copy(qg[0:64, :], qf[64:128, :])
                nc.vector.tensor_copy(qg[64:128, :], qf[0:64, :])
                m1 = rtp.tile([128, CH], f16, tag="qf", name="m1")
                m2 = rtp.tile([128, CH], f16, tag="qf", name="m2")
                nc.vector.tensor_mul(m1[:], qf[:], trigC_sb[:, c * CH:(c + 1) * CH])
                nc.vector.tensor_mul(m2[:], qg[:], trigD_sb[:, c * CH:(c + 1) * CH])
                nc.vector.tensor_add(dst[:], m1[:], m2[:])

            qts_all = [None] * NCH

            def proj_rope(c):
                # K first (scores need it earliest), then V, then Q heads.
                kt_ps = psMM.tile([128, CH], f32, tag="mm", name=f"ktps{c}")
                for k in range(NK):
                    nc.tensor.matmul(
                        kt_ps[:], wk_sb[:, k * HD:(k + 1) * HD], xts[c][:, k * CH:(k + 1) * CH],
                        start=(k == 0), stop=(k == NK - 1))
                rope(kt_sb[:, c * CH:(c + 1) * CH], kt_ps[:], c)

                vt_ps = psMM.tile([128, CH], f32, tag="mm", name=f"vtps{c}")
                for k in range(NK):
                    nc.tensor.matmul(
                        vt_ps[:], wv_sb[:, k * HD:(k + 1) * HD], xts[c][:, k * CH:(k + 1) * CH],
                        start=(k == 0), stop=(k == NK - 1))
                vt_sb = vsp.tile([128, CH], f16, tag="vt", name=f"vt{c}")
                nc.scalar.copy(vt_sb[:], vt_ps[:])
                for g in range(4):
                    jt = 4 * c + g
                    nc.scalar.dma_start_transpose(
                        v_sb[:, jt * 128:(jt + 1) * 128],
                        vt_sb[:, g * 128:(g + 1) * 128])

                qts = []
                for h in range(HQ):
                    qt_ps = psMM.tile([128, CH], f32, tag="mm", name=f"qtps{c}_{h}")
                    for k in range(NK):
                        nc.tensor.matmul(
                            qt_ps[:],
                            wq_sb[:, k * QD + h * HD: k * QD + (h + 1) * HD],
                            xts[c][:, k * CH:(k + 1) * CH],
                            start=(k == 0), stop=(k == NK - 1))
                    qr = qtp.tile([128, CH], f16, tag="qt", name=f"qr{c}_{h}")
                    rope(qr, qt_ps[:], c)
                    qts.append(qr)
                qts_all[c] = qts

            ats_all = [None] * NCH

            def attention(c):
                jts = list(range(max(0, 4 * c - 8), 4 * c + 4))
                L = len(jts)
                ats = []
                for h in range(HQ):
                    pvt = psPV.tile([128, CH], f32, tag="pv",
                                    name=f"pv{c}_{h}")
                    dnt = psPV.tile([128, CH], f32, tag="dn",
                                    name=f"dn{c}_{h}")
                    pv_ps = pvt[:]
                    dn_ps = dnt[:]
                    pts = [None] * ((L + 1) // 2)

                    def pv_dn(i):
                        ph = pts[i // 2][:, (i % 2) * CH:(i % 2 + 1) * CH]
                        nc.tensor.matmul(
                            pv_ps, v_sb[:, jts[i] * 128:(jts[i] + 1) * 128],
                            ph, start=(i == 0), stop=(i == L - 1))
                        nc.tensor.matmul(
                            dn_ps, one_sb[:], ph,
                            start=(i == 0), stop=(i == L - 1))

                    stp = [None]
                    for i, jt in enumerate(jts):
                        if i % 2 == 0:
                            pts[i // 2] = ptp.tile([128, 2 * CH], f16, tag="pt",
                                                   name=f"pt{c}_{h}_{i}")
                            stp[0] = psMM.tile([128, 2 * CH], f32, tag="mm",
                                               name=f"st{c}_{h}_{i}")
                        pt = pts[i // 2]
                        st = stp[0]
                        nc.tensor.matmul(
                            st[:, (i % 2) * CH:(i % 2 + 1) * CH],
                            kt_sb[:, jt * 128:(jt + 1) * 128],
                            qts_all[c][h][:], start=True, stop=True)
                        if i % 2 == 1:
                            nc.scalar.activation(pt[:], st[:], EXP)
                            offA = 128 * jts[i - 1] - CH * c
                            if offA >= 0:
                                # diagonal: keep iff il >= jl + offA + 128*t
                                nc.gpsimd.affine_select(
                                    out=pt[:], in_=pt[:],
                                    pattern=[[-128, 2], [1, CH]],
                                    compare_op=GE, fill=0.0,
                                    base=-offA, channel_multiplier=-1)
                            elif offA <= -(CH + 2):
                                # window edge: keep iff il <= jl + offA+128t + W
                                nc.gpsimd.affine_select(
                                    out=pt[:], in_=pt[:],
                                    pattern=[[128, 2], [-1, CH]],
                                    compare_op=GE, fill=0.0,
                                    base=offA + WINDOW, channel_multiplier=1)
                        if i >= 2:
                            pv_dn(i - 2)
                    pv_dn(L - 2)
                    pv_dn(L - 1)

                    rbc = smp.tile([128, CH], f32, tag="rbc", name=f"rbc{c}_{h}")
                    nc.vector.reciprocal_approx_fast(out=rbc[:], in_=dn_ps)
                    at = atp.tile([128, CH], f16, tag="at", name=f"at{c}_{h}")
                    nc.vector.tensor_tensor(at[:], pv_ps, rbc[:], MUL)
                    ats.append(at)
                ats_all[c] = ats

            def oproj(c):
                ats = ats_all[c]
                for g in range(4):
                    ob = obp.tile([128, D], f16, tag="ob", name=f"ob{c}_{g}")
                    for q in range(4):
                        ecol = q * CH
                        op_ = psPV.tile([128, CH], f32, tag="pv",
                                        name=f"op{c}_{g}{q}")
                        for h in range(HQ):
                            nc.tensor.matmul(
                                op_[:],
                                ats[h][:, g * 128:(g + 1) * 128],
                                wo_sb[:, h * D + ecol: h * D + ecol + CH],
                                start=(h == 0), stop=(h == HQ - 1))
                        if q % 2 == 0:
                            nc.scalar.copy(ob[:, ecol:ecol + CH], op_[:])
                        else:
                            nc.vector.tensor_copy(ob[:, ecol:ecol + CH], op_[:])
                        if q % 2 == 1:
                            nc.sync.dma_start(
                                out_d[c * CH + g * 128: c * CH + (g + 1) * 128,
                                      (q - 1) * CH:(q + 1) * CH],
                                ob[:, (q - 1) * CH:(q + 1) * CH])

            for t in range(NCH + 1):
                if t >= 1:
                    attention(t - 1)
                if t < NCH:
                    proj_rope(t)
                if t >= 1:
                    oproj(t - 1)

    nc.finalize()
    return nc


def _get_nc():
    if "nc" not in _cache:
        _cache["nc"] = build_kernel()
    return _cache["nc"]


def kernel(x, wq, wk, wv, wo):
    from concourse.bass_utils import run_bass_kernel_spmd

    x = np.asarray(x, dtype=np.float32)
    shards, trigC, trigD = _host_prep(
        np.asarray(wq, np.float32), np.asarray(wk, np.float32),
        np.asarray(wv, np.float32), np.asarray(wo, np.float32))

    ones = np.ones((128, 128), dtype=np.float16)
    xTs = [np.ascontiguousarray(x[b].T).astype(np.float16) for b in range(B)]

    in_maps = []
    for c in range(8):
        b = c // 4
        m = dict(shards[c])
        m.update(xT=xTs[b], trigC=trigC, trigD=trigD, ones=ones)
        in_maps.append(m)

    nc = _get_nc()
    res = run_bass_kernel_spmd(
        nc, in_maps, core_ids=list(range(8)),
        trace=bool(int(os.environ.get("KERNEL_TRACE", "0"))),
    )
    _cache["last_result"] = res
    parts = [r["out"] for r in res.results]
    out = np.empty((B, S, D), dtype=np.float32)
    for b in range(B):
        out[b] = (parts[4 * b].astype(np.float32)
                  + parts[4 * b + 1].astype(np.float32)
                  + parts[4 * b + 2].astype(np.float32)
                  + parts[4 * b + 3].astype(np.float32))
    return out


# revision 15
# speedup vs baseline: 1.0601x; 1.0601x over previous
"""Trainium2 Bass kernel for a local-attention layer (GQA + RoPE + banded mask).

Full computation (reference semantics, f32):
  q = x@wq, k = x@wk, v = x@wv  (B=2, S=2048, D=2048, Hq=16, Hkv=4, hd=128)
  rope(q), rope(k) interleaved-pair style
  banded causal attention, window=1024, softmax
  out = (probs @ v_rep) @ wo

Sharding: 8 cores = (batch b in {0,1}) x (kv-group g in {0..3}).
Core c handles batch c//4, kv head c%4 and its 4 q heads.  Each core
computes a partial (2048, 2048) f16 output (its heads' contribution
through wo rows); host sums the 4 partials per batch in f32.

v1 design (all matmuls f16, f32 PSUM accum; PE kept warm):
  - X^T is precomputed on the HOST (f16), so no PE transposes are needed
    for the projections; QT/KT/VT = W^T X^T directly.
  - V natural layout obtained via DMA-xbar transpose (scalar-engine HWDGE),
    not the PE.
  - RoPE in half-split form (host pre-permutes wq/wk columns); computed
    with 4 vector ops per tile using two trig tables [cos;sin] and
    [sin;cos].
  - Scores computed transposed ST[j,i] = KT_tile^T @ QT as (128,512)
    singles; exp on ScalarE (no max subtraction -- scores are provably
    small); banded mask applied post-exp on f16 tiles with pair-merged
    gpsimd.affine_select (one call per homogeneous pair).
  - Softmax denominator via ones-vector matmul into the second PSUM bank
    of the per-head pvdn tile; reciprocal_approx_fast + partition
    broadcast; normalization fused into PSUM->SBUF copy of pv.
  - o-proj accumulates over the 4 heads; f16 output DMA.
  - PSUM plan (8 banks, all tiles single-bank (128,512) -- two-bank
    tiles measurably slow PE writes): tag mm x2 [proj], tag pv x2
    [attention pv + oproj], tag dn x2, tag st x2 [scores].
  - Chunk loop is software-pipelined [attn(c-1)][proj(c)][oproj(c-1)] so
    the PE never sits idle during softmax post-processing.
"""

import os
import numpy as np

B, S, D = 2, 2048, 2048
NH, NKV, HD = 16, 4, 128
WINDOW = 1024
ROPE_THETA = 10000.0
HQ = NH // NKV          # q heads per core = 4
QD = HQ * HD            # 512
NK = D // 128           # 16 contraction chunks
CH = 512                # s-chunk size
NCH = S // CH           # 4 chunks

_cache = {}


def _host_prep(wq, wk, wv, wo):
    """Per-core weight slices with rope permutation + scale folded in."""
    # de-interleave permutation: dev col j <- ref col (2j if j<64 else 2(j-64)+1)
    perm = np.empty(HD, dtype=np.int64)
    perm[:64] = np.arange(64) * 2
    perm[64:] = np.arange(64) * 2 + 1

    scale = 1.0 / np.sqrt(np.float32(HD))
    wq_p = wq.reshape(D, NH, HD)[:, :, perm].reshape(D, NH * HD) * scale
    wk_p = wk.reshape(D, NKV, HD)[:, :, perm].reshape(D, NKV * HD)

    inv_freq = 1.0 / (ROPE_THETA ** (np.arange(0, HD, 2, dtype=np.float32) / HD))
    t = np.arange(S, dtype=np.float32)
    ang = np.outer(t, inv_freq)             # (S, 64)
    cosT = np.ascontiguousarray(np.cos(ang).T).astype(np.float32)  # (64, S)
    sinT = np.ascontiguousarray(np.sin(ang).T).astype(np.float32)
    # trigC = [cos; cos], trigD = [-sin; sin]: with qf=[lo;hi], qg=[hi;lo],
    # rope(q) = qf*trigC + qg*trigD (every op pairs equal base partitions).
    trigC = np.ascontiguousarray(np.concatenate([cosT, cosT], axis=0)).astype(np.float16)
    trigD = np.ascontiguousarray(np.concatenate([-sinT, sinT], axis=0)).astype(np.float16)

    shards = []
    for c in range(8):
        g = c % 4
        shards.append(dict(
            wq=np.ascontiguousarray(wq_p[:, g * QD:(g + 1) * QD]).astype(np.float16),
            wk=np.ascontiguousarray(wk_p[:, g * HD:(g + 1) * HD]).astype(np.float16),
            wv=np.ascontiguousarray(wv[:, g * HD:(g + 1) * HD]).astype(np.float16),
            wo=np.ascontiguousarray(wo[g * QD:(g + 1) * QD, :]).astype(np.float16),
        ))
    return shards, trigC, trigD


def build_kernel():
    import concourse.bass as bass
    import concourse.mybir as mybir
    import concourse.tile as tile
    from concourse import bacc

    f16 = mybir.dt.float16
    f32 = mybir.dt.float32
    EXP = mybir.ActivationFunctionType.Exp
    GE = mybir.AluOpType.is_ge
    MUL = mybir.AluOpType.mult

    nc = bacc.Bacc("TRN2", target_bir_lowering=False, debug=False, num_devices=8)

    xt_d = nc.dram_tensor("xT", [D, S], f16, kind="ExternalInput").ap()
    wq_d = nc.dram_tensor("wq", [D, QD], f16, kind="ExternalInput").ap()
    wk_d = nc.dram_tensor("wk", [D, HD], f16, kind="ExternalInput").ap()
    wv_d = nc.dram_tensor("wv", [D, HD], f16, kind="ExternalInput").ap()
    wo_d = nc.dram_tensor("wo", [QD, D], f16, kind="ExternalInput").ap()
    trigC_d = nc.dram_tensor("trigC", [128, S], f16, kind="ExternalInput").ap()
    trigD_d = nc.dram_tensor("trigD", [128, S], f16, kind="ExternalInput").ap()
    one_d = nc.dram_tensor("ones", [128, 128], f16, kind="ExternalInput").ap()
    out_d = nc.dram_tensor("out", [S, D], f16, kind="ExternalOutput").ap()

    with tile.TileContext(nc) as tc:
        with (
            tc.tile_pool(name="persist", bufs=1) as pp,
            tc.tile_pool(name="xtp", bufs=NCH) as xp,
            tc.tile_pool(name="qtp", bufs=6) as qtp,
            tc.tile_pool(name="vsp", bufs=2) as vsp,
            tc.tile_pool(name="ptp", bufs=3) as ptp,
            tc.tile_pool(name="ropet", bufs=8) as rtp,
            tc.tile_pool(name="small", bufs=3) as smp,
            tc.tile_pool(name="atp", bufs=5) as atp,
            tc.tile_pool(name="obp", bufs=2) as obp,
            tc.tile_pool(name="psMM", bufs=2, space="PSUM") as psMM,
            tc.tile_pool(name="psPV", bufs=2, space="PSUM") as psPV,
            tc.tile_pool(name="psST", bufs=2, space="PSUM") as psST,
        ):
            # ---- persistent SBUF tensors -------------------------------
            wq_sb = pp.tile([128, NK * QD], f16, tag="wq")      # [k][:, qd]
            wk_sb = pp.tile([128, NK * HD], f16, tag="wk")
            wv_sb = pp.tile([128, NK * HD], f16, tag="wv")
            wo_sb = pp.tile([128, HQ * D], f16, tag="wo")       # [h][:, e]
            trigC_sb = pp.tile([128, S], f16, tag="trigC")  # [cos;cos]
            trigD_sb = pp.tile([128, S], f16, tag="trigD")  # [-sin;sin]
            one_sb = pp.tile([128, 128], f16, tag="one")
            kt_sb = pp.tile([128, S], f16, tag="kt")            # rope'd K^T
            v_sb = pp.tile([128, S], f16, tag="v")              # [jt][s, d]

            # ---- input DMAs (sync engine, in consumption order) --------
            nc.sync.dma_start(one_sb[:], one_d)
            nc.sync.dma_start(trigC_sb[:], trigC_d)
            nc.sync.dma_start(
                wk_sb[:].rearrange("p (k n) -> p k n", k=NK),
                wk_d.rearrange("(k p) n -> p k n", p=128))

            xts = [None] * NCH

            def load_x_chunk(c):
                t_ = xp.tile([128, NK * CH], f16, tag="xt", name=f"xt{c}")
                for q in range(4):
                    nc.sync.dma_start(
                        t_[:, q * 4 * CH:(q + 1) * 4 * CH].rearrange(
                            "p (k s) -> p k s", k=4),
                        xt_d[q * 4 * 128:(q + 1) * 4 * 128,
                             c * CH:(c + 1) * CH].rearrange(
                            "(k p) s -> p k s", p=128))
                xts[c] = t_

            load_x_chunk(0)
            nc.sync.dma_start(trigD_sb[:], trigD_d)
            nc.sync.dma_start(
                wv_sb[:].rearrange("p (k n) -> p k n", k=NK),
                wv_d.rearrange("(k p) n -> p k n", p=128))
            nc.sync.dma_start(
                wq_sb[:].rearrange("p (k n) -> p k n", k=NK),
                wq_d.rearrange("(k p) n -> p k n", p=128))
            load_x_chunk(1)
            nc.sync.dma_start(
                wo_sb[:].rearrange("p (h n) -> p h n", h=HQ),
                wo_d.rearrange("(h p) n -> p h n", p=128))
            load_x_chunk(2)
            load_x_chunk(3)

            # PE warm-up: build HAM activity while the first x/w DMAs land
            # so the real matmuls start at K=8/8 instead of 1.2 GHz.
            warm_ps = psST.tile([128, CH], f32, tag="st", name="warmup")
            for _ in range(11):
                nc.tensor.matmul(warm_ps[:], one_sb[:], trigC_sb[:, 0:CH],
                                 start=True, stop=True)

            def rope(dst, src_ps, c):
                """src_ps (128, CH) psum -> dst (128, CH), half-split rope:
                dst = [lo*cos - hi*sin ; lo*sin + hi*cos].
                qf=[lo;hi] (scalar copy, frees the PSUM slot early), then
                qg=[hi;lo] via two single-input copies; all two-input ops
                pair equal base partitions and run f16 2x on the DVE."""
                qf = rtp.tile([128, CH], f16, tag="qf", name="qf")
                nc.vector.tensor_copy(qf[:], src_ps)
                qg = rtp.tile([128, CH], f16, tag="qf", name="qg")
                nc.vector.tensor_copy(qg[0:64, :], qf[64:128, :])
                nc.vector.tensor_copy(qg[64:128, :], qf[0:64, :])
                m1 = rtp.tile([128, CH], f16, tag="qf", name="m1")
                m2 = rtp.tile([128, CH], f16, tag="qf", name="m2")
                nc.vector.tensor_mul(m1[:], qf[:], trigC_sb[:, c * CH:(c + 1) * CH])
                nc.vector.tensor_mul(m2[:], qg[:], trigD_sb[:, c * CH:(c + 1) * CH])
                nc.vector.tensor_add(dst[:], m1[:], m2[:])

            qts_all = [None] * NCH

            def proj_rope(c):
                # K first (scores need it earliest), then V, then Q heads.
                kt_ps = psMM.tile([128, CH], f32, tag="mm", name=f"ktps{c}")
                for k in range(NK):
                    nc.tensor.matmul(
                        kt_ps[:], wk_sb[:, k * HD:(k + 1) * HD], xts[c][:, k * CH:(k + 1) * CH],
                        start=(k == 0), stop=(k == NK - 1))
                rope(kt_sb[:, c * CH:(c + 1) * CH], kt_ps[:], c)

                vt_ps = psMM.tile([128, CH], f32, tag="mm", name=f"vtps{c}")
                for k in range(NK):
                    nc.tensor.matmul(
                        vt_ps[:], wv_sb[:, k * HD:(k + 1) * HD], xts[c][:, k * CH:(k + 1) * CH],
                        start=(k == 0), stop=(k == NK - 1))
                vt_sb = vsp.tile([128, CH], f16, tag="vt", name=f"vt{c}")
                nc.scalar.copy(vt_sb[:], vt_ps[:])
                for g in range(4):
                    jt = 4 * c + g
                    nc.scalar.dma_start_transpose(
                        v_sb[:, jt * 128:(jt + 1) * 128],
                        vt_sb[:, g * 128:(g + 1) * 128])

                qts = []
                for h in range(HQ):
                    qt_ps = psMM.tile([128, CH], f32, tag="mm", name=f"qtps{c}_{h}")
                    for k in range(NK):
                        nc.tensor.matmul(
                            qt_ps[:],
                            wq_sb[:, k * QD + h * HD: k * QD + (h + 1) * HD],
                            xts[c][:, k * CH:(k + 1) * CH],
                            start=(k == 0), stop=(k == NK - 1))
                    qr = qtp.tile([128, CH], f16, tag="qt", name=f"qr{c}_{h}")
                    rope(qr, qt_ps[:], c)
                    qts.append(qr)
                qts_all[c] = qts

            ats_all = [None] * NCH

            def attention(c):
                jts = list(range(max(0, 4 * c - 8), 4 * c + 4))
                L = len(jts)
                ats = []
                for h in range(HQ):
                    pvt = psPV.tile([128, CH], f32, tag="pv",
                                    name=f"pv{c}_{h}")
                    dnt = psPV.tile([128, CH], f32, tag="dn",
                                    name=f"dn{c}_{h}")
                    pv_ps = pvt[:]
                    dn_ps = dnt[:]
                    pts = [None] * ((L + 1) // 2)

                    def pv_dn(i):
                        ph = pts[i // 2][:, (i % 2) * CH:(i % 2 + 1) * CH]
                        nc.tensor.matmul(
                            pv_ps, v_sb[:, jts[i] * 128:(jts[i] + 1) * 128],
                            ph, start=(i == 0), stop=(i == L - 1))
                        nc.tensor.matmul(
                            dn_ps, one_sb[:], ph,
                            start=(i == 0), stop=(i == L - 1))

                    for i, jt in enumerate(jts):
                        if i % 2 == 0:
                            pts[i // 2] = ptp.tile([128, 2 * CH], f16, tag="pt",
                                                   name=f"pt{c}_{h}_{i}")
                        pt = pts[i // 2]
                        st = psST.tile([128, CH], f32, tag="st",
                                       name=f"st{c}_{h}_{i}")
                        nc.tensor.matmul(
                            st[:], kt_sb[:, jt * 128:(jt + 1) * 128],
                            qts_all[c][h][:], start=True, stop=True)
                        nc.scalar.activation(
                            pt[:, (i % 2) * CH:(i % 2 + 1) * CH], st[:], EXP)
                        if i % 2 == 1:
                            offA = 128 * jts[i - 1] - CH * c
                            if offA >= 0:
                                # diagonal: keep iff il >= jl + offA + 128*t
                                nc.gpsimd.affine_select(
                                    out=pt[:], in_=pt[:],
                                    pattern=[[-128, 2], [1, CH]],
                                    compare_op=GE, fill=0.0,
                                    base=-offA, channel_multiplier=-1)
                            elif offA <= -(CH + 2):
                                # window edge: keep iff il <= jl + offA+128t + W
                                nc.gpsimd.affine_select(
                                    out=pt[:], in_=pt[:],
                                    pattern=[[128, 2], [-1, CH]],
                                    compare_op=GE, fill=0.0,
                                    base=offA + WINDOW, channel_multiplier=1)
                        if i >= 2:
                            pv_dn(i - 2)
                    pv_dn(L - 2)
                    pv_dn(L - 1)

                    rbc = smp.tile([128, CH], f32, tag="rbc", name=f"rbc{c}_{h}")
                    nc.vector.reciprocal_approx_fast(out=rbc[:], in_=dn_ps)
                    at = atp.tile([128, CH], f16, tag="at", name=f"at{c}_{h}")
                    nc.vector.tensor_tensor(at[:], pv_ps, rbc[:], MUL)
                    ats.append(at)
                ats_all[c] = ats

            def oproj(c):
                ats = ats_all[c]
                for g in range(4):
                    ob = obp.tile([128, D], f16, tag="ob", name=f"ob{c}_{g}")
                    for q in range(4):
                        ecol = q * CH
                        op_ = psPV.tile([128, CH], f32, tag="pv",
                                        name=f"op{c}_{g}{q}")
                        for h in range(HQ):
                            nc.tensor.matmul(
                                op_[:],
                                ats[h][:, g * 128:(g + 1) * 128],
                                wo_sb[:, h * D + ecol: h * D + ecol + CH],
                                start=(h == 0), stop=(h == HQ - 1))
                        if q % 2 == 0:
                            nc.scalar.copy(ob[:, ecol:ecol + CH], op_[:])
                        else:
                            nc.vector.tensor_copy(ob[:, ecol:ecol + CH], op_[:])
                        if q % 2 == 1:
                            nc.sync.dma_start(
                                out_d[c * CH + g * 128: c * CH + (g + 1) * 128,
                                      (q - 1) * CH:(q + 1) * CH],
                                ob[:, (q - 1) * CH:(q + 1) * CH])

            for t in range(NCH + 1):
                if t >= 1:
                    attention(t - 1)
                if t < NCH:
                    proj_rope(t)
                if t >= 1:
                    oproj(t - 1)

    nc.finalize()
    return nc


def _get_nc():
    if "nc" not in _cache:
        _cache["nc"] = build_kernel()
    return _cache["nc"]


def kernel(x, wq, wk, wv, wo):
    from concourse.bass_utils import run_bass_kernel_spmd

    x = np.asarray(x, dtype=np.float32)
    shards, trigC, trigD = _host_prep(
        np.asarray(wq, np.float32), np.asarray(wk, np.float32),
        np.asarray(wv, np.float32), np.asarray(wo, np.float32))

    ones = np.ones((128, 128), dtype=np.float16)
    xTs = [np.ascontiguousarray(x[b].T).astype(np.float16) for b in range(B)]

    in_maps = []
    for c in range(8):
        b = c // 4
        m = dict(shards[c])
        m.update(xT=xTs[b], trigC=trigC, trigD=trigD, ones=ones)
        in_maps.append(m)

    nc = _get_nc()
    res = run_bass_kernel_spmd(
        nc, in_maps, core_ids=list(range(8)),
        trace=bool(int(os.environ.get("KERNEL_TRACE", "0"))),
    )
    _cache["last_result"] = res
    parts = [r["out"] for r in res.results]
    out = np.empty((B, S, D), dtype=np.float32)
    for b in range(B):
        out[b] = (parts[4 * b].astype(np.float32)
                  + parts[4 * b + 1].astype(np.float32)
                  + parts[4 * b + 2].astype(np.float32)
                  + parts[4 * b + 3].astype(np.float32))
    return out


# revision 16
# speedup vs baseline: 1.0873x; 1.0257x over previous
"""Trainium2 Bass kernel for a local-attention layer (GQA + RoPE + banded mask).

Full computation (reference semantics, f32):
  q = x@wq, k = x@wk, v = x@wv  (B=2, S=2048, D=2048, Hq=16, Hkv=4, hd=128)
  rope(q), rope(k) interleaved-pair style
  banded causal attention, window=1024, softmax
  out = (probs @ v_rep) @ wo

Sharding: 8 cores = (batch b in {0,1}) x (kv-group g in {0..3}).
Core c handles batch c//4, kv head c%4 and its 4 q heads.  Each core
computes a partial (2048, 2048) f16 output (its heads' contribution
through wo rows); host sums the 4 partials per batch in f32.

v1 design (all matmuls f16, f32 PSUM accum; PE kept warm):
  - X^T is precomputed on the HOST (f16), so no PE transposes are needed
    for the projections; QT/KT/VT = W^T X^T directly.
  - V natural layout obtained via DMA-xbar transpose (scalar-engine HWDGE),
    not the PE.
  - RoPE in half-split form (host pre-permutes wq/wk columns); computed
    with 4 vector ops per tile using two trig tables [cos;sin] and
    [sin;cos].
  - Scores computed transposed ST[j,i] = KT_tile^T @ QT as (128,512)
    singles; exp on ScalarE (no max subtraction -- scores are provably
    small); banded mask applied post-exp on f16 tiles with pair-merged
    gpsimd.affine_select (one call per homogeneous pair).
  - Softmax denominator via ones-vector matmul into the second PSUM bank
    of the per-head pvdn tile; reciprocal_approx_fast + partition
    broadcast; normalization fused into PSUM->SBUF copy of pv.
  - o-proj accumulates over the 4 heads; f16 output DMA.
  - PSUM plan (8 banks, all tiles single-bank (128,512) -- two-bank
    tiles measurably slow PE writes): tag mm x2 [proj], tag pv x2
    [attention pv + oproj], tag dn x2, tag st x2 [scores].
  - Chunk loop is software-pipelined [attn(c-1)][proj(c)][oproj(c-1)] so
    the PE never sits idle during softmax post-processing.
"""

import os
import numpy as np

B, S, D = 2, 2048, 2048
NH, NKV, HD = 16, 4, 128
WINDOW = 1024
ROPE_THETA = 10000.0
HQ = NH // NKV          # q heads per core = 4
QD = HQ * HD            # 512
NK = D // 128           # 16 contraction chunks
CH = 512                # s-chunk size
NCH = S // CH           # 4 chunks

_cache = {}


def _host_prep(wq, wk, wv, wo):
    """Per-core weight slices with rope permutation + scale folded in."""
    # de-interleave permutation: dev col j <- ref col (2j if j<64 else 2(j-64)+1)
    perm = np.empty(HD, dtype=np.int64)
    perm[:64] = np.arange(64) * 2
    perm[64:] = np.arange(64) * 2 + 1

    scale = 1.0 / np.sqrt(np.float32(HD))
    wq_p = wq.reshape(D, NH, HD)[:, :, perm].reshape(D, NH * HD) * scale
    wk_p = wk.reshape(D, NKV, HD)[:, :, perm].reshape(D, NKV * HD)

    inv_freq = 1.0 / (ROPE_THETA ** (np.arange(0, HD, 2, dtype=np.float32) / HD))
    t = np.arange(S, dtype=np.float32)
    ang = np.outer(t, inv_freq)             # (S, 64)
    cosT = np.ascontiguousarray(np.cos(ang).T).astype(np.float32)  # (64, S)
    sinT = np.ascontiguousarray(np.sin(ang).T).astype(np.float32)
    # trigC = [cos; cos], trigD = [-sin; sin]: with qf=[lo;hi], qg=[hi;lo],
    # rope(q) = qf*trigC + qg*trigD (every op pairs equal base partitions).
    trigC = np.ascontiguousarray(np.concatenate([cosT, cosT], axis=0)).astype(np.float16)
    trigD = np.ascontiguousarray(np.concatenate([-sinT, sinT], axis=0)).astype(np.float16)

    shards = []
    for c in range(8):
        g = c % 4
        shards.append(dict(
            wq=np.ascontiguousarray(wq_p[:, g * QD:(g + 1) * QD]).astype(np.float16),
            wk=np.ascontiguousarray(wk_p[:, g * HD:(g + 1) * HD]).astype(np.float16),
            wv=np.ascontiguousarray(wv[:, g * HD:(g + 1) * HD]).astype(np.float16),
            wo=np.ascontiguousarray(wo[g * QD:(g + 1) * QD, :]).astype(np.float16),
        ))
    return shards, trigC, trigD


def build_kernel():
    import concourse.bass as bass
    import concourse.mybir as mybir
    import concourse.tile as tile
    from concourse import bacc

    f16 = mybir.dt.float16
    f32 = mybir.dt.float32
    EXP = mybir.ActivationFunctionType.Exp
    GE = mybir.AluOpType.is_ge
    MUL = mybir.AluOpType.mult

    nc = bacc.Bacc("TRN2", target_bir_lowering=False, debug=False, num_devices=8)

    xt_d = nc.dram_tensor("xT", [D, S], f16, kind="ExternalInput").ap()
    wq_d = nc.dram_tensor("wq", [D, QD], f16, kind="ExternalInput").ap()
    wk_d = nc.dram_tensor("wk", [D, HD], f16, kind="ExternalInput").ap()
    wv_d = nc.dram_tensor("wv", [D, HD], f16, kind="ExternalInput").ap()
    wo_d = nc.dram_tensor("wo", [QD, D], f16, kind="ExternalInput").ap()
    trigC_d = nc.dram_tensor("trigC", [128, S], f16, kind="ExternalInput").ap()
    trigD_d = nc.dram_tensor("trigD", [128, S], f16, kind="ExternalInput").ap()
    one_d = nc.dram_tensor("ones", [128, 128], f16, kind="ExternalInput").ap()
    out_d = nc.dram_tensor("out", [S, D], f16, kind="ExternalOutput").ap()

    with tile.TileContext(nc) as tc:
        with (
            tc.tile_pool(name="persist", bufs=1) as pp,
            tc.tile_pool(name="xtp", bufs=NCH) as xp,
            tc.tile_pool(name="qtp", bufs=6) as qtp,
            tc.tile_pool(name="vsp", bufs=2) as vsp,
            tc.tile_pool(name="ptp", bufs=3) as ptp,
            tc.tile_pool(name="ropet", bufs=8) as rtp,
            tc.tile_pool(name="small", bufs=3) as smp,
            tc.tile_pool(name="atp", bufs=5) as atp,
            tc.tile_pool(name="obp", bufs=2) as obp,
            tc.tile_pool(name="psMM", bufs=2, space="PSUM") as psMM,
            tc.tile_pool(name="psPV", bufs=2, space="PSUM") as psPV,
            tc.tile_pool(name="psST", bufs=2, space="PSUM") as psST,
        ):
            # ---- persistent SBUF tensors -------------------------------
            wq_sb = pp.tile([128, NK * QD], f16, tag="wq")      # [k][:, qd]
            wk_sb = pp.tile([128, NK * HD], f16, tag="wk")
            wv_sb = pp.tile([128, NK * HD], f16, tag="wv")
            wo_sb = pp.tile([128, HQ * D], f16, tag="wo")       # [h][:, e]
            trigC_sb = pp.tile([128, S], f16, tag="trigC")  # [cos;cos]
            trigD_sb = pp.tile([128, S], f16, tag="trigD")  # [-sin;sin]
            one_sb = pp.tile([128, 128], f16, tag="one")
            kt_sb = pp.tile([128, S], f16, tag="kt")            # rope'd K^T
            v_sb = pp.tile([128, S], f16, tag="v")              # [jt][s, d]

            # ---- input DMAs (sync engine, in consumption order) --------
            nc.sync.dma_start(one_sb[:], one_d)
            nc.sync.dma_start(trigC_sb[:], trigC_d)
            nc.sync.dma_start(
                wk_sb[:].rearrange("p (k n) -> p k n", k=NK),
                wk_d.rearrange("(k p) n -> p k n", p=128))

            xts = [None] * NCH

            def load_x_chunk(c):
                t_ = xp.tile([128, NK * CH], f16, tag="xt", name=f"xt{c}")
                for q in range(4):
                    nc.sync.dma_start(
                        t_[:, q * 4 * CH:(q + 1) * 4 * CH].rearrange(
                            "p (k s) -> p k s", k=4),
                        xt_d[q * 4 * 128:(q + 1) * 4 * 128,
                             c * CH:(c + 1) * CH].rearrange(
                            "(k p) s -> p k s", p=128))
                xts[c] = t_

            load_x_chunk(0)
            nc.sync.dma_start(trigD_sb[:], trigD_d)
            nc.sync.dma_start(
                wv_sb[:].rearrange("p (k n) -> p k n", k=NK),
                wv_d.rearrange("(k p) n -> p k n", p=128))
            nc.sync.dma_start(
                wq_sb[:].rearrange("p (k n) -> p k n", k=NK),
                wq_d.rearrange("(k p) n -> p k n", p=128))
            load_x_chunk(1)
            nc.sync.dma_start(
                wo_sb[:].rearrange("p (h n) -> p h n", h=HQ),
                wo_d.rearrange("(h p) n -> p h n", p=128))
            load_x_chunk(2)
            load_x_chunk(3)

            # PE warm-up: build HAM activity while the first x/w DMAs land
            # so the real matmuls start at K=8/8 instead of 1.2 GHz.
            warm_ps = psST.tile([128, CH], f32, tag="st", name="warmup")
            for _ in range(11):
                nc.tensor.matmul(warm_ps[:], one_sb[:], trigC_sb[:, 0:CH],
                                 start=True, stop=True)

            def rope(dst, src_ps, c):
                """src_ps (128, CH) psum -> dst (128, CH), half-split rope:
                dst = [lo*cos - hi*sin ; lo*sin + hi*cos].
                qf=[lo;hi] (scalar copy, frees the PSUM slot early), then
                qg=[hi;lo] via two single-input copies; all two-input ops
                pair equal base partitions and run f16 2x on the DVE."""
                qf = rtp.tile([128, CH], f16, tag="qf", name="qf")
                nc.vector.tensor_copy(qf[:], src_ps)
                qg = rtp.tile([128, CH], f16, tag="qf", name="qg")
                nc.vector.tensor_copy(qg[0:64, :], qf[64:128, :])
                nc.vector.tensor_copy(qg[64:128, :], qf[0:64, :])
                m1 = rtp.tile([128, CH], f16, tag="qf", name="m1")
                m2 = rtp.tile([128, CH], f16, tag="qf", name="m2")
                nc.vector.tensor_mul(m1[:], qf[:], trigC_sb[:, c * CH:(c + 1) * CH])
                nc.vector.tensor_mul(m2[:], qg[:], trigD_sb[:, c * CH:(c + 1) * CH])
                nc.vector.tensor_add(dst[:], m1[:], m2[:])

            qts_all = [None] * NCH

            def proj_rope(c):
                # K first (scores need it earliest), then V, then Q heads.
                kt_ps = psMM.tile([128, CH], f32, tag="mm", name=f"ktps{c}")
                for k in range(NK):
                    nc.tensor.matmul(
                        kt_ps[:], wk_sb[:, k * HD:(k + 1) * HD], xts[c][:, k * CH:(k + 1) * CH],
                        start=(k == 0), stop=(k == NK - 1))
                rope(kt_sb[:, c * CH:(c + 1) * CH], kt_ps[:], c)

                vt_ps = psMM.tile([128, CH], f32, tag="mm", name=f"vtps{c}")
                for k in range(NK):
                    nc.tensor.matmul(
                        vt_ps[:], wv_sb[:, k * HD:(k + 1) * HD], xts[c][:, k * CH:(k + 1) * CH],
                        start=(k == 0), stop=(k == NK - 1))
                vt_sb = vsp.tile([128, CH], f16, tag="vt", name=f"vt{c}")
                nc.scalar.copy(vt_sb[:], vt_ps[:])
                for g in range(4):
                    jt = 4 * c + g
                    nc.scalar.dma_start_transpose(
                        v_sb[:, jt * 128:(jt + 1) * 128],
                        vt_sb[:, g * 128:(g + 1) * 128])

                qts = []
                for h in range(HQ):
                    qt_ps = psMM.tile([128, CH], f32, tag="mm", name=f"qtps{c}_{h}")
                    for k in range(NK):
                        nc.tensor.matmul(
                            qt_ps[:],
                            wq_sb[:, k * QD + h * HD: k * QD + (h + 1) * HD],
                            xts[c][:, k * CH:(k + 1) * CH],
                            start=(k == 0), stop=(k == NK - 1))
                    qr = qtp.tile([128, CH], f16, tag="qt", name=f"qr{c}_{h}")
                    rope(qr, qt_ps[:], c)
                    qts.append(qr)
                qts_all[c] = qts

            ats_all = [None] * NCH

            def attention(c):
                jts = list(range(max(0, 4 * c - 8), 4 * c + 4))
                L = len(jts)
                ats = []
                for h in range(HQ):
                    pvt = psPV.tile([128, CH], f32, tag="pv",
                                    name=f"pv{c}_{h}")
                    dnt = psPV.tile([128, CH], f32, tag="dn",
                                    name=f"dn{c}_{h}")
                    pv_ps = pvt[:]
                    dn_ps = dnt[:]
                    pts = [None] * ((L + 1) // 2)

                    def irange(jt):
                        # extreme banded tiles have one fully-masked i-half
                        off = 128 * jt - CH * c
                        if off <= -896:
                            return 0, 256
                        if off >= 256:
                            return 256, CH
                        return 0, CH

                    def pv_dn(i):
                        jt = jts[i]
                        ilo, ihi = irange(jt)
                        ph = pts[i // 2][:, (i % 2) * CH + ilo:
                                         (i % 2) * CH + ihi]
                        nc.tensor.matmul(
                            pv_ps[:, ilo:ihi],
                            v_sb[:, jt * 128:(jt + 1) * 128],
                            ph, start=(i == 0), stop=(i == L - 1))
                        nc.tensor.matmul(
                            dn_ps[:, ilo:ihi], one_sb[:], ph,
                            start=(i == 0), stop=(i == L - 1))

                    for i, jt in enumerate(jts):
                        if i % 2 == 0:
                            pts[i // 2] = ptp.tile([128, 2 * CH], f16, tag="pt",
                                                   name=f"pt{c}_{h}_{i}")
                        pt = pts[i // 2]
                        ilo, ihi = irange(jt)
                        w = ihi - ilo
                        st = psST.tile([128, CH], f32, tag="st",
                                       name=f"st{c}_{h}_{i}")
                        nc.tensor.matmul(
                            st[:, 0:w], kt_sb[:, jt * 128:(jt + 1) * 128],
                            qts_all[c][h][:, ilo:ihi], start=True, stop=True)
                        nc.scalar.activation(
                            pt[:, (i % 2) * CH + ilo:(i % 2) * CH + ihi],
                            st[:, 0:w], EXP)
                        if i % 2 == 1:
                            offA = 128 * jts[i - 1] - CH * c
                            if offA == -1024:
                                # edge halves, live il in [0,256): one select
                                # per tile: keep il <= jl + offX + W
                                for q_, offX in ((0, -1024), (1, -896)):
                                    nc.gpsimd.affine_select(
                                        out=pt[:, q_ * CH:q_ * CH + 256],
                                        in_=pt[:, q_ * CH:q_ * CH + 256],
                                        pattern=[[-1, 256]],
                                        compare_op=GE, fill=0.0,
                                        base=offX + WINDOW,
                                        channel_multiplier=1)
                            elif offA == 256:
                                # diag halves, live il in [256,512):
                                # keep 256+s >= jl + offX
                                for q_, offX in ((0, 256), (1, 384)):
                                    nc.gpsimd.affine_select(
                                        out=pt[:, q_ * CH + 256:(q_ + 1) * CH],
                                        in_=pt[:, q_ * CH + 256:(q_ + 1) * CH],
                                        pattern=[[1, 256]],
                                        compare_op=GE, fill=0.0,
                                        base=256 - offX,
                                        channel_multiplier=-1)
                            elif offA == 0:
                                # full-width diagonal pair (merged)
                                nc.gpsimd.affine_select(
                                    out=pt[:], in_=pt[:],
                                    pattern=[[-128, 2], [1, CH]],
                                    compare_op=GE, fill=0.0,
                                    base=-offA, channel_multiplier=-1)
                            elif offA == -768:
                                # full-width window-edge pair (merged)
                                nc.gpsimd.affine_select(
                                    out=pt[:], in_=pt[:],
                                    pattern=[[128, 2], [-1, CH]],
                                    compare_op=GE, fill=0.0,
                                    base=offA + WINDOW, channel_multiplier=1)
                        if i >= 2:
                            pv_dn(i - 2)
                    pv_dn(L - 2)
                    pv_dn(L - 1)

                    rbc = smp.tile([128, CH], f32, tag="rbc", name=f"rbc{c}_{h}")
                    nc.vector.reciprocal_approx_fast(out=rbc[:], in_=dn_ps)
                    at = atp.tile([128, CH], f16, tag="at", name=f"at{c}_{h}")
                    nc.vector.tensor_tensor(at[:], pv_ps, rbc[:], MUL)
                    ats.append(at)
                ats_all[c] = ats

            def oproj(c):
                ats = ats_all[c]
                for g in range(4):
                    ob = obp.tile([128, D], f16, tag="ob", name=f"ob{c}_{g}")
                    for q in range(4):
                        ecol = q * CH
                        op_ = psPV.tile([128, CH], f32, tag="pv",
                                        name=f"op{c}_{g}{q}")
                        for h in range(HQ):
                            nc.tensor.matmul(
                                op_[:],
                                ats[h][:, g * 128:(g + 1) * 128],
                                wo_sb[:, h * D + ecol: h * D + ecol + CH],
                                start=(h == 0), stop=(h == HQ - 1))
                        if q % 2 == 0:
                            nc.scalar.copy(ob[:, ecol:ecol + CH], op_[:])
                        else:
                            nc.vector.tensor_copy(ob[:, ecol:ecol + CH], op_[:])
                        if q % 2 == 1:
                            nc.sync.dma_start(
                                out_d[c * CH + g * 128: c * CH + (g + 1) * 128,
                                      (q - 1) * CH:(q + 1) * CH],
                                ob[:, (q - 1) * CH:(q + 1) * CH])

            for t in range(NCH + 1):
                if t >= 1:
                    attention(t - 1)
                if t < NCH:
                    proj_rope(t)
                if t >= 1:
                    oproj(t - 1)

    nc.finalize()
    return nc


def _get_nc():
    if "nc" not in _cache:
        _cache["nc"] = build_kernel()
    return _cache["nc"]


def kernel(x, wq, wk, wv, wo):
    from concourse.bass_utils import run_bass_kernel_spmd

    x = np.asarray(x, dtype=np.float32)
    shards, trigC, trigD = _host_prep(
        np.asarray(wq, np.float32), np.asarray(wk, np.float32),
        np.asarray(wv, np.float32), np.asarray(wo, np.float32))

    ones = np.ones((128, 128), dtype=np.float16)
    xTs = [np.ascontiguousarray(x[b].T).astype(np.float16) for b in range(B)]

    in_maps = []
    for c in range(8):
        b = c // 4
        m = dict(shards[c])
        m.update(xT=xTs[b], trigC=trigC, trigD=trigD, ones=ones)
        in_maps.append(m)

    nc = _get_nc()
    res = run_bass_kernel_spmd(
        nc, in_maps, core_ids=list(range(8)),
        trace=bool(int(os.environ.get("KERNEL_TRACE", "0"))),
    )
    _cache["last_result"] = res
    parts = [r["out"] for r in res.results]
    out = np.empty((B, S, D), dtype=np.float32)
    for b in range(B):
        out[b] = (parts[4 * b].astype(np.float32)
                  + parts[4 * b + 1].astype(np.float32)
                  + parts[4 * b + 2].astype(np.float32)
                  + parts[4 * b + 3].astype(np.float32))
    return out


# revision 17
# speedup vs baseline: 1.1126x; 1.0233x over previous
"""Trainium2 Bass kernel for a local-attention layer (GQA + RoPE + banded mask).

Full computation (reference semantics, f32):
  q = x@wq, k = x@wk, v = x@wv  (B=2, S=2048, D=2048, Hq=16, Hkv=4, hd=128)
  rope(q), rope(k) interleaved-pair style
  banded causal attention, window=1024, softmax
  out = (probs @ v_rep) @ wo

Sharding: 8 cores = (batch b in {0,1}) x (kv-group g in {0..3}).
Core c handles batch c//4, kv head c%4 and its 4 q heads.  Each core
computes a partial (2048, 2048) f16 output (its heads' contribution
through wo rows); host sums the 4 partials per batch in f32.

v1 design (all matmuls f16, f32 PSUM accum; PE kept warm):
  - X^T is precomputed on the HOST (f16), so no PE transposes are needed
    for the projections; QT/KT/VT = W^T X^T directly.
  - V natural layout obtained via DMA-xbar transpose (scalar-engine HWDGE),
    not the PE.
  - RoPE in half-split form (host pre-permutes wq/wk columns); computed
    with 4 vector ops per tile using two trig tables [cos;sin] and
    [sin;cos].
  - Scores computed transposed ST[j,i] = KT_tile^T @ QT as (128,512)
    singles; exp on ScalarE (no max subtraction -- scores are provably
    small); banded mask applied post-exp on f16 tiles with pair-merged
    gpsimd.affine_select (one call per homogeneous pair).
  - Softmax denominator via ones-vector matmul into the second PSUM bank
    of the per-head pvdn tile; reciprocal_approx_fast + partition
    broadcast; normalization fused into PSUM->SBUF copy of pv.
  - o-proj accumulates over the 4 heads; f16 output DMA.
  - PSUM plan (8 banks, all tiles single-bank (128,512) -- two-bank
    tiles measurably slow PE writes): tag mm x2 [proj], tag pv x2
    [attention pv + oproj], tag dn x2, tag st x2 [scores].
  - Chunk loop is software-pipelined [attn(c-1)][proj(c)][oproj(c-1)] so
    the PE never sits idle during softmax post-processing.
"""

import os
import numpy as np

B, S, D = 2, 2048, 2048
NH, NKV, HD = 16, 4, 128
WINDOW = 1024
ROPE_THETA = 10000.0
HQ = NH // NKV          # q heads per core = 4
QD = HQ * HD            # 512
NK = D // 128           # 16 contraction chunks
CH = 512                # s-chunk size
NCH = S // CH           # 4 chunks

_cache = {}


def _host_prep(wq, wk, wv, wo):
    """Per-core weight slices with rope permutation + scale folded in."""
    # de-interleave permutation: dev col j <- ref col (2j if j<64 else 2(j-64)+1)
    perm = np.empty(HD, dtype=np.int64)
    perm[:64] = np.arange(64) * 2
    perm[64:] = np.arange(64) * 2 + 1

    scale = 1.0 / np.sqrt(np.float32(HD))
    wq_p = wq.reshape(D, NH, HD)[:, :, perm].reshape(D, NH * HD) * scale
    wk_p = wk.reshape(D, NKV, HD)[:, :, perm].reshape(D, NKV * HD)

    inv_freq = 1.0 / (ROPE_THETA ** (np.arange(0, HD, 2, dtype=np.float32) / HD))
    t = np.arange(S, dtype=np.float32)
    ang = np.outer(t, inv_freq)             # (S, 64)
    cosT = np.ascontiguousarray(np.cos(ang).T).astype(np.float32)  # (64, S)
    sinT = np.ascontiguousarray(np.sin(ang).T).astype(np.float32)
    # trigC = [cos; cos], trigD = [-sin; sin]: with qf=[lo;hi], qg=[hi;lo],
    # rope(q) = qf*trigC + qg*trigD (every op pairs equal base partitions).
    trigC = np.ascontiguousarray(np.concatenate([cosT, cosT], axis=0)).astype(np.float16)
    trigD = np.ascontiguousarray(np.concatenate([-sinT, sinT], axis=0)).astype(np.float16)

    shards = []
    for c in range(8):
        g = c % 4
        shards.append(dict(
            wq=np.ascontiguousarray(wq_p[:, g * QD:(g + 1) * QD]).astype(np.float16),
            wk=np.ascontiguousarray(wk_p[:, g * HD:(g + 1) * HD]).astype(np.float16),
            wv=np.ascontiguousarray(wv[:, g * HD:(g + 1) * HD]).astype(np.float16),
            wo=np.ascontiguousarray(wo[g * QD:(g + 1) * QD, :]).astype(np.float16),
        ))
    return shards, trigC, trigD


def build_kernel():
    import concourse.bass as bass
    import concourse.mybir as mybir
    import concourse.tile as tile
    from concourse import bacc

    f16 = mybir.dt.float16
    f32 = mybir.dt.float32
    EXP = mybir.ActivationFunctionType.Exp
    GE = mybir.AluOpType.is_ge
    MUL = mybir.AluOpType.mult

    nc = bacc.Bacc("TRN2", target_bir_lowering=False, debug=False, num_devices=8)

    xt_d = nc.dram_tensor("xT", [D, S], f16, kind="ExternalInput").ap()
    wq_d = nc.dram_tensor("wq", [D, QD], f16, kind="ExternalInput").ap()
    wk_d = nc.dram_tensor("wk", [D, HD], f16, kind="ExternalInput").ap()
    wv_d = nc.dram_tensor("wv", [D, HD], f16, kind="ExternalInput").ap()
    wo_d = nc.dram_tensor("wo", [QD, D], f16, kind="ExternalInput").ap()
    trigC_d = nc.dram_tensor("trigC", [128, S], f16, kind="ExternalInput").ap()
    trigD_d = nc.dram_tensor("trigD", [128, S], f16, kind="ExternalInput").ap()
    one_d = nc.dram_tensor("ones", [128, 128], f16, kind="ExternalInput").ap()
    out_d = nc.dram_tensor("out", [S, D], f16, kind="ExternalOutput").ap()

    with tile.TileContext(nc) as tc:
        with (
            tc.tile_pool(name="persist", bufs=1) as pp,
            tc.tile_pool(name="xtp", bufs=NCH) as xp,
            tc.tile_pool(name="qtp", bufs=6) as qtp,
            tc.tile_pool(name="vsp", bufs=2) as vsp,
            tc.tile_pool(name="ptp", bufs=3) as ptp,
            tc.tile_pool(name="ropet", bufs=8) as rtp,
            tc.tile_pool(name="small", bufs=3) as smp,
            tc.tile_pool(name="atp", bufs=5) as atp,
            tc.tile_pool(name="obp", bufs=2) as obp,
            tc.tile_pool(name="psMM", bufs=2, space="PSUM") as psMM,
            tc.tile_pool(name="psPV", bufs=2, space="PSUM") as psPV,
            tc.tile_pool(name="psST", bufs=2, space="PSUM") as psST,
        ):
            # ---- persistent SBUF tensors -------------------------------
            wq_sb = pp.tile([128, NK * QD], f16, tag="wq")      # [k][:, qd]
            wk_sb = pp.tile([128, NK * HD], f16, tag="wk")
            wv_sb = pp.tile([128, NK * HD], f16, tag="wv")
            wo_sb = pp.tile([128, HQ * D], f16, tag="wo")       # [h][:, e]
            trigC_sb = pp.tile([128, S], f16, tag="trigC")  # [cos;cos]
            trigD_sb = pp.tile([128, S], f16, tag="trigD")  # [-sin;sin]
            one_sb = pp.tile([128, 128], f16, tag="one")
            kt_sb = pp.tile([128, S], f16, tag="kt")            # rope'd K^T
            v_sb = pp.tile([128, S], f16, tag="v")              # [jt][s, d]

            # ---- input DMAs (sync engine, in consumption order) --------
            nc.sync.dma_start(one_sb[:], one_d)
            nc.sync.dma_start(trigC_sb[:], trigC_d)
            nc.sync.dma_start(
                wk_sb[:].rearrange("p (k n) -> p k n", k=NK),
                wk_d.rearrange("(k p) n -> p k n", p=128))

            xts = [None] * NCH

            def load_x_chunk(c):
                t_ = xp.tile([128, NK * CH], f16, tag="xt", name=f"xt{c}")
                for q in range(4):
                    nc.sync.dma_start(
                        t_[:, q * 4 * CH:(q + 1) * 4 * CH].rearrange(
                            "p (k s) -> p k s", k=4),
                        xt_d[q * 4 * 128:(q + 1) * 4 * 128,
                             c * CH:(c + 1) * CH].rearrange(
                            "(k p) s -> p k s", p=128))
                xts[c] = t_

            load_x_chunk(0)
            nc.sync.dma_start(trigD_sb[:], trigD_d)
            nc.sync.dma_start(
                wv_sb[:].rearrange("p (k n) -> p k n", k=NK),
                wv_d.rearrange("(k p) n -> p k n", p=128))
            nc.sync.dma_start(
                wq_sb[:].rearrange("p (k n) -> p k n", k=NK),
                wq_d.rearrange("(k p) n -> p k n", p=128))
            load_x_chunk(1)
            nc.sync.dma_start(
                wo_sb[:].rearrange("p (h n) -> p h n", h=HQ),
                wo_d.rearrange("(h p) n -> p h n", p=128))
            load_x_chunk(2)
            load_x_chunk(3)

            # PE warm-up: build HAM activity while the first x/w DMAs land
            # so the real matmuls start at K=8/8 instead of 1.2 GHz.
            warm_ps = psST.tile([128, CH], f32, tag="st", name="warmup")
            for _ in range(11):
                nc.tensor.matmul(warm_ps[:], one_sb[:], trigC_sb[:, 0:CH],
                                 start=True, stop=True)

            def rope(dst, src_ps, c):
                """src_ps (128, CH) psum -> dst (128, CH), half-split rope:
                dst = [lo*cos - hi*sin ; lo*sin + hi*cos].
                qf=[lo;hi] (scalar copy, frees the PSUM slot early), then
                qg=[hi;lo] via two single-input copies; all two-input ops
                pair equal base partitions and run f16 2x on the DVE."""
                qf = rtp.tile([128, CH], f16, tag="qf", name="qf")
                nc.vector.tensor_copy(qf[:], src_ps)
                qg = rtp.tile([128, CH], f16, tag="qf", name="qg")
                nc.vector.tensor_copy(qg[0:64, :], qf[64:128, :])
                nc.vector.tensor_copy(qg[64:128, :], qf[0:64, :])
                m1 = rtp.tile([128, CH], f16, tag="qf", name="m1")
                m2 = rtp.tile([128, CH], f16, tag="qf", name="m2")
                nc.vector.tensor_mul(m1[:], qf[:], trigC_sb[:, c * CH:(c + 1) * CH])
                nc.vector.tensor_mul(m2[:], qg[:], trigD_sb[:, c * CH:(c + 1) * CH])
                nc.vector.tensor_add(dst[:], m1[:], m2[:])

            qts_all = [None] * NCH

            def proj_rope(c):
                # K first (scores need it earliest), then V, then Q heads.
                kt_ps = psMM.tile([128, CH], f32, tag="mm", name=f"ktps{c}")
                for k in range(NK):
                    nc.tensor.matmul(
                        kt_ps[:], wk_sb[:, k * HD:(k + 1) * HD], xts[c][:, k * CH:(k + 1) * CH],
                        start=(k == 0), stop=(k == NK - 1))
                rope(kt_sb[:, c * CH:(c + 1) * CH], kt_ps[:], c)

                vt_ps = psMM.tile([128, CH], f32, tag="mm", name=f"vtps{c}")
                for k in range(NK):
                    nc.tensor.matmul(
                        vt_ps[:], wv_sb[:, k * HD:(k + 1) * HD], xts[c][:, k * CH:(k + 1) * CH],
                        start=(k == 0), stop=(k == NK - 1))
                vt_sb = vsp.tile([128, CH], f16, tag="vt", name=f"vt{c}")
                nc.scalar.copy(vt_sb[:], vt_ps[:])
                for g in range(4):
                    jt = 4 * c + g
                    nc.scalar.dma_start_transpose(
                        v_sb[:, jt * 128:(jt + 1) * 128],
                        vt_sb[:, g * 128:(g + 1) * 128])

                qts = []
                for h in range(HQ):
                    qt_ps = psMM.tile([128, CH], f32, tag="mm", name=f"qtps{c}_{h}")
                    for k in range(NK):
                        nc.tensor.matmul(
                            qt_ps[:],
                            wq_sb[:, k * QD + h * HD: k * QD + (h + 1) * HD],
                            xts[c][:, k * CH:(k + 1) * CH],
                            start=(k == 0), stop=(k == NK - 1))
                    qr = qtp.tile([128, CH], f16, tag="qt", name=f"qr{c}_{h}")
                    rope(qr, qt_ps[:], c)
                    qts.append(qr)
                qts_all[c] = qts

            ats_all = [None] * NCH

            def attention(c):
                jts = list(range(max(0, 4 * c - 8), 4 * c + 4))
                L = len(jts)
                ats = []
                for h in range(HQ):
                    pvt = psPV.tile([128, CH], f32, tag="pv",
                                    name=f"pv{c}_{h}")
                    dnt = psPV.tile([128, CH], f32, tag="dn",
                                    name=f"dn{c}_{h}")
                    pv_ps = pvt[:]
                    dn_ps = dnt[:]
                    pts = [None] * ((L + 1) // 2)

                    def irange(jt):
                        # banded tiles: drop fully-masked 128-col i-quarters
                        # (masks still zero any garbage in the live slices)
                        off = 128 * jt - CH * c
                        live = {-1024: (0, 128), -896: (0, 256),
                                -768: (0, 384), 128: (128, CH),
                                256: (256, CH), 384: (384, CH)}
                        return live.get(off, (0, CH))

                    def pv_dn(i):
                        jt = jts[i]
                        ilo, ihi = irange(jt)
                        ph = pts[i // 2][:, (i % 2) * CH + ilo:
                                         (i % 2) * CH + ihi]
                        nc.tensor.matmul(
                            pv_ps[:, ilo:ihi],
                            v_sb[:, jt * 128:(jt + 1) * 128],
                            ph, start=(i == 0), stop=(i == L - 1))
                        nc.tensor.matmul(
                            dn_ps[:, ilo:ihi], one_sb[:], ph,
                            start=(i == 0), stop=(i == L - 1))

                    for i, jt in enumerate(jts):
                        if i % 2 == 0:
                            pts[i // 2] = ptp.tile([128, 2 * CH], f16, tag="pt",
                                                   name=f"pt{c}_{h}_{i}")
                        pt = pts[i // 2]
                        ilo, ihi = irange(jt)
                        w = ihi - ilo
                        st = psST.tile([128, CH], f32, tag="st",
                                       name=f"st{c}_{h}_{i}")
                        nc.tensor.matmul(
                            st[:, 0:w], kt_sb[:, jt * 128:(jt + 1) * 128],
                            qts_all[c][h][:, ilo:ihi], start=True, stop=True)
                        nc.scalar.activation(
                            pt[:, (i % 2) * CH + ilo:(i % 2) * CH + ihi],
                            st[:, 0:w], EXP)
                        if i % 2 == 1:
                            offA = 128 * jts[i - 1] - CH * c
                            if offA == -1024:
                                # edge halves, live il in [0,256): one select
                                # per tile: keep il <= jl + offX + W
                                for q_, offX in ((0, -1024), (1, -896)):
                                    nc.gpsimd.affine_select(
                                        out=pt[:, q_ * CH:q_ * CH + 256],
                                        in_=pt[:, q_ * CH:q_ * CH + 256],
                                        pattern=[[-1, 256]],
                                        compare_op=GE, fill=0.0,
                                        base=offX + WINDOW,
                                        channel_multiplier=1)
                            elif offA == 256:
                                # diag halves, live il in [256,512):
                                # keep 256+s >= jl + offX
                                for q_, offX in ((0, 256), (1, 384)):
                                    nc.gpsimd.affine_select(
                                        out=pt[:, q_ * CH + 256:(q_ + 1) * CH],
                                        in_=pt[:, q_ * CH + 256:(q_ + 1) * CH],
                                        pattern=[[1, 256]],
                                        compare_op=GE, fill=0.0,
                                        base=256 - offX,
                                        channel_multiplier=-1)
                            elif offA == 0:
                                # full-width diagonal pair (merged)
                                nc.gpsimd.affine_select(
                                    out=pt[:], in_=pt[:],
                                    pattern=[[-128, 2], [1, CH]],
                                    compare_op=GE, fill=0.0,
                                    base=-offA, channel_multiplier=-1)
                            elif offA == -768:
                                # full-width window-edge pair (merged)
                                nc.gpsimd.affine_select(
                                    out=pt[:], in_=pt[:],
                                    pattern=[[128, 2], [-1, CH]],
                                    compare_op=GE, fill=0.0,
                                    base=offA + WINDOW, channel_multiplier=1)
                        if i >= 2:
                            pv_dn(i - 2)
                    pv_dn(L - 2)
                    pv_dn(L - 1)

                    rbc = smp.tile([128, CH], f32, tag="rbc", name=f"rbc{c}_{h}")
                    nc.vector.reciprocal_approx_fast(out=rbc[:], in_=dn_ps)
                    at = atp.tile([128, CH], f16, tag="at", name=f"at{c}_{h}")
                    nc.vector.tensor_tensor(at[:], pv_ps, rbc[:], MUL)
                    ats.append(at)
                ats_all[c] = ats

            def oproj(c):
                ats = ats_all[c]
                for g in range(4):
                    ob = obp.tile([128, D], f16, tag="ob", name=f"ob{c}_{g}")
                    for q in range(4):
                        ecol = q * CH
                        op_ = psPV.tile([128, CH], f32, tag="pv",
                                        name=f"op{c}_{g}{q}")
                        for h in range(HQ):
                            nc.tensor.matmul(
                                op_[:],
                                ats[h][:, g * 128:(g + 1) * 128],
                                wo_sb[:, h * D + ecol: h * D + ecol + CH],
                                start=(h == 0), stop=(h == HQ - 1))
                        if q % 2 == 0:
                            nc.scalar.copy(ob[:, ecol:ecol + CH], op_[:])
                        else:
                            nc.vector.tensor_copy(ob[:, ecol:ecol + CH], op_[:])
                        if q % 2 == 1:
                            nc.sync.dma_start(
                                out_d[c * CH + g * 128: c * CH + (g + 1) * 128,
                                      (q - 1) * CH:(q + 1) * CH],
                                ob[:, (q - 1) * CH:(q + 1) * CH])

            for t in range(NCH + 1):
                if t >= 1:
                    attention(t - 1)
                if t < NCH:
                    proj_rope(t)
                if t >= 1:
                    oproj(t - 1)

    nc.finalize()
    return nc


def _get_nc():
    if "nc" not in _cache:
        _cache["nc"] = build_kernel()
    return _cache["nc"]


def kernel(x, wq, wk, wv, wo):
    from concourse.bass_utils import run_bass_kernel_spmd

    x = np.asarray(x, dtype=np.float32)
    shards, trigC, trigD = _host_prep(
        np.asarray(wq, np.float32), np.asarray(wk, np.float32),
        np.asarray(wv, np.float32), np.asarray(wo, np.float32))

    ones = np.ones((128, 128), dtype=np.float16)
    xTs = [np.ascontiguousarray(x[b].T).astype(np.float16) for b in range(B)]

    in_maps = []
    for c in range(8):
        b = c // 4
        m = dict(shards[c])
        m.update(xT=xTs[b], trigC=trigC, trigD=trigD, ones=ones)
        in_maps.append(m)

    nc = _get_nc()
    res = run_bass_kernel_spmd(
        nc, in_maps, core_ids=list(range(8)),
        trace=bool(int(os.environ.get("KERNEL_TRACE", "0"))),
    )
    _cache["last_result"] = res
    parts = [r["out"] for r in res.results]
    out = np.empty((B, S, D), dtype=np.float32)
    for b in range(B):
        out[b] = (parts[4 * b].astype(np.float32)
                  + parts[4 * b + 1].astype(np.float32)
                  + parts[4 * b + 2].astype(np.float32)
                  + parts[4 * b + 3].astype(np.float32))
    return out
